# revision 2
# baseline (speedup 1.0000x reference)
"""Trainium2 Bass kernel for nn_BatchLinear (segmented path-indexed grouped linear, MoE-routed).

Math (per token b with expert e = w_id[b], 8 paths (i, j, k, alpha)):
    out[b, 128*k:+128] += alpha * x[b, 128*i:+128] @ W[e, seg j]  (each seg 128x128)

Strategy:
  - Host: route tokens by expert (the "all-to-all token dispatch"), pack each
    core's tokens feature-major ([expert, feature, token]) so the device
    matmuls need no transpose, prescale W segs 4-7 by the path coefficient 0.5.
  - Device (8 cores, data parallel, weights replicated): for each expert block
    and token tile, 8 fp32r matmuls accumulate the 4 output segments in PSUM
    (2 paths per output segment), copy to SBUF, DMA out.
  - Host: scatter rows back to original token order.
"""

import numpy as np

import concourse.bacc as bacc
import concourse.mybir as mybir
import concourse.tile as tile
from concourse.bass_utils import run_bass_kernel_spmd

N_CORES = 8
B = 32768
E = 4
U = V = 128
IN_STRIDE = 512
N_SEG = 4  # input/output feature segments
# out seg k <- (input seg, weight seg) x 2 contributions (coefficients folded
# into the prescaled weights: segs 4-7 are scaled by 0.5 on the host)
CONTRIB = {0: [(0, 0), (3, 7)], 1: [(1, 1), (0, 4)], 2: [(2, 2), (1, 5)], 3: [(3, 3), (2, 6)]}

F32 = mybir.dt.float32
F32R = mybir.dt.float32r

_cache = {}


def _token_tiles(cap):
    tiles = []
    t0 = 0
    while t0 < cap:
        T = min(512, cap - t0)
        tiles.append((t0, T))
        t0 += T
    return tiles


def _build(cap):
    """Build + schedule the per-core Bass program for per-(core,expert) capacity `cap`."""
    if cap in _cache:
        return _cache[cap]

    nc = bacc.Bacc("TRN2", target_bir_lowering=False, debug=False, num_devices=N_CORES)
    x = nc.dram_tensor("x", [E, IN_STRIDE, cap], F32, kind="ExternalInput")
    w = nc.dram_tensor("w", [E, 8, U, V], F32, kind="ExternalInput")
    y = nc.dram_tensor("y", [E, IN_STRIDE, cap], F32, kind="ExternalOutput")

    with tile.TileContext(nc) as tc:
        with (
            tc.tile_pool(name="wpool", bufs=1) as wp,
            tc.tile_pool(name="xin", bufs=3) as xp,
            tc.tile_pool(name="yout", bufs=3) as yp,
            tc.tile_pool(name="ps", bufs=2, space="PSUM") as pp,
        ):
            # all expert weights resident in SBUF: [u, (e j), v]
            wt = wp.tile([U, E * 8, V], F32R)
            nc.sync.dma_start(wt[:], w[:].rearrange("e j u v -> u (e j) v").bitcast(F32R))

            for e in range(E):
                for (t0, T) in _token_tiles(cap):
                    xs = xp.tile([128, N_SEG, T], F32R, tag="xs")
                    nc.sync.dma_start(
                        xs[:],
                        x[e, :, t0 : t0 + T]
                        .rearrange("(s p) t -> p s t", p=128)
                        .bitcast(F32R),
                    )
                    ys = yp.tile([128, N_SEG, T], F32, tag="ys")
                    for k in range(N_SEG):
                        ps = pp.tile([128, T], F32, tag=f"ps{k}")
                        (i1, j1), (i2, j2) = CONTRIB[k]
                        nc.tensor.matmul(
                            ps[:],
                            wt[:, e * 8 + j1, :],
                            xs[:, i1, :],
                            start=True,
                            stop=False,
                        )
                        nc.tensor.matmul(
                            ps[:],
                            wt[:, e * 8 + j2, :],
                            xs[:, i2, :],
                            start=False,
                            stop=True,
                        )
                        nc.vector.tensor_copy(ys[:, k, :], ps[:])
                    nc.sync.dma_start(
                        y[e, :, t0 : t0 + T].rearrange("(s p) t -> p s t", p=128), ys[:]
                    )

    nc.compile()
    _cache[cap] = nc
    return nc


def _route(tensor_w_id):
    """Split each expert's tokens into N_CORES chunks. Returns (chunks, cap):
    chunks[c][e] = 1-D array of token indices for core c, expert e."""
    chunks = [[None] * E for _ in range(N_CORES)]
    max_n = 1
    for e in range(E):
        idx_e = np.flatnonzero(tensor_w_id == e)
        parts = np.array_split(idx_e, N_CORES)
        for c in range(N_CORES):
            chunks[c][e] = parts[c]
            max_n = max(max_n, len(parts[c]))
    cap = -(-max_n // 64) * 64
    return chunks, cap


def _run(tensor_in, tensor_w, tensor_w_id, trace=False):
    tensor_in = np.ascontiguousarray(tensor_in, dtype=np.float32)
    tensor_w = np.asarray(tensor_w, dtype=np.float32)
    tensor_w_id = np.asarray(tensor_w_id, dtype=np.int32)

    chunks, cap = _route(tensor_w_id)
    nc = _build(cap)

    # prescale: fold the 0.5 path coefficient into weight segs 4-7
    w_pack = tensor_w.reshape(E, 8, U, V).copy()
    w_pack[:, 4:] *= 0.5

    # pack: gather + transpose to [E, feature, token] per core
    big_idx = np.zeros((N_CORES, E, cap), dtype=np.int64)
    for c in range(N_CORES):
        for e in range(E):
            idx = chunks[c][e]
            big_idx[c, e, : len(idx)] = idx
    xg = tensor_in[big_idx.reshape(-1)]  # [N_CORES*E*cap, 512]
    xg = xg.reshape(N_CORES, E, cap, IN_STRIDE).transpose(0, 1, 3, 2)  # -> [c, e, f, t]

    in_maps = [
        {"x": np.ascontiguousarray(xg[c]), "w": w_pack} for c in range(N_CORES)
    ]
    res = run_bass_kernel_spmd(nc, in_maps, list(range(N_CORES)), trace=trace)

    out = np.empty((B, IN_STRIDE), dtype=np.float32)
    for c in range(N_CORES):
        yc = res.results[c]["y"]  # [E, 512, cap]
        for e in range(E):
            idx = chunks[c][e]
            if len(idx):
                out[idx] = yc[e, :, : len(idx)].T
    return out, res


def kernel(tensor_in, tensor_w, tensor_w_id):
    out, _ = _run(tensor_in, tensor_w, tensor_w_id)
    return out


# revision 3
# speedup vs baseline: 1.0602x; 1.0602x over previous
"""Trainium2 Bass kernel for nn_BatchLinear (segmented path-indexed grouped linear, MoE-routed).

Math (per token b with expert e = w_id[b], 8 paths (i, j, k, alpha)):
    out[b, 128*k:+128] += alpha * x[b, 128*i:+128] @ W[e, seg j]  (each seg 128x128)

Strategy:
  - Host: route tokens by expert (the "all-to-all token dispatch"), pack each
    core's tokens feature-major ([expert, feature, token]) so the device
    matmuls need no transpose, prescale W segs 4-7 by the path coefficient 0.5.
  - Device (8 cores, data parallel, weights replicated): for each expert block
    and token tile, 8 fp32r matmuls accumulate the 4 output segments in PSUM
    (2 paths per output segment), copy to SBUF, DMA out.
  - Host: scatter rows back to original token order.
"""

import numpy as np

import concourse.bacc as bacc
import concourse.mybir as mybir
import concourse.tile as tile
from concourse.bass_utils import run_bass_kernel_spmd

N_CORES = 8
B = 32768
E = 4
U = V = 128
IN_STRIDE = 512
N_SEG = 4  # input/output feature segments
# out seg k <- (input seg, weight seg) x 2 contributions (coefficients folded
# into the prescaled weights: segs 4-7 are scaled by 0.5 on the host)
CONTRIB = {0: [(0, 0), (3, 7)], 1: [(1, 1), (0, 4)], 2: [(2, 2), (1, 5)], 3: [(3, 3), (2, 6)]}

F32 = mybir.dt.float32
F32R = mybir.dt.float32r

_cache = {}


def _token_tiles(cap):
    tiles = []
    t0 = 0
    while t0 < cap:
        T = min(512, cap - t0)
        tiles.append((t0, T))
        t0 += T
    return tiles


def _build(cap):
    """Build + schedule the per-core Bass program for per-(core,expert) capacity `cap`."""
    if cap in _cache:
        return _cache[cap]

    nc = bacc.Bacc("TRN2", target_bir_lowering=False, debug=False, num_devices=N_CORES)
    x = nc.dram_tensor("x", [E, IN_STRIDE, cap], F32, kind="ExternalInput")
    w = nc.dram_tensor("w", [E, 8, U, V], F32, kind="ExternalInput")
    y = nc.dram_tensor("y", [E, IN_STRIDE, cap], F32, kind="ExternalOutput")

    with tile.TileContext(nc) as tc:
        with (
            tc.tile_pool(name="wpool", bufs=1) as wp,
            tc.tile_pool(name="xin", bufs=2) as xp,
            tc.tile_pool(name="yout", bufs=2) as yp,
            tc.tile_pool(name="ps", bufs=2, space="PSUM") as pp,
        ):
            # all expert weights resident in SBUF: [u, (e j), v]
            wt = wp.tile([U, E * 8, V], F32R)
            nc.sync.dma_start(wt[:], w[:].rearrange("e j u v -> u (e j) v").bitcast(F32R))

            for e in range(E):
                # whole expert block in one DMA: [p, seg, token] <- [seg*128, cap]
                xs = xp.tile([128, N_SEG, cap], F32R, tag="xs")
                nc.sync.dma_start(
                    xs[:], x[e].rearrange("(s p) t -> p s t", p=128).bitcast(F32R)
                )
                ys = yp.tile([128, N_SEG, cap], F32, tag="ys")
                for (t0, T) in _token_tiles(cap):
                    for k in range(N_SEG):
                        ps = pp.tile([128, T], F32, tag=f"ps{k}")
                        (i1, j1), (i2, j2) = CONTRIB[k]
                        nc.tensor.matmul(
                            ps[:],
                            wt[:, e * 8 + j1, :],
                            xs[:, i1, t0 : t0 + T],
                            start=True,
                            stop=False,
                        )
                        nc.tensor.matmul(
                            ps[:],
                            wt[:, e * 8 + j2, :],
                            xs[:, i2, t0 : t0 + T],
                            start=False,
                            stop=True,
                        )
                        nc.vector.tensor_copy(ys[:, k, t0 : t0 + T], ps[:])
                nc.sync.dma_start(
                    y[e].rearrange("(s p) t -> p s t", p=128), ys[:]
                )

    nc.compile()
    _cache[cap] = nc
    return nc


def _route(tensor_w_id):
    """Split each expert's tokens into N_CORES chunks. Returns (chunks, cap):
    chunks[c][e] = 1-D array of token indices for core c, expert e."""
    chunks = [[None] * E for _ in range(N_CORES)]
    max_n = 1
    for e in range(E):
        idx_e = np.flatnonzero(tensor_w_id == e)
        parts = np.array_split(idx_e, N_CORES)
        for c in range(N_CORES):
            chunks[c][e] = parts[c]
            max_n = max(max_n, len(parts[c]))
    cap = -(-max_n // 64) * 64
    return chunks, cap


def _run(tensor_in, tensor_w, tensor_w_id, trace=False):
    tensor_in = np.ascontiguousarray(tensor_in, dtype=np.float32)
    tensor_w = np.asarray(tensor_w, dtype=np.float32)
    tensor_w_id = np.asarray(tensor_w_id, dtype=np.int32)

    chunks, cap = _route(tensor_w_id)
    nc = _build(cap)

    # prescale: fold the 0.5 path coefficient into weight segs 4-7
    w_pack = tensor_w.reshape(E, 8, U, V).copy()
    w_pack[:, 4:] *= 0.5

    # pack: gather + transpose to [E, feature, token] per core
    big_idx = np.zeros((N_CORES, E, cap), dtype=np.int64)
    for c in range(N_CORES):
        for e in range(E):
            idx = chunks[c][e]
            big_idx[c, e, : len(idx)] = idx
    xg = tensor_in[big_idx.reshape(-1)]  # [N_CORES*E*cap, 512]
    xg = xg.reshape(N_CORES, E, cap, IN_STRIDE).transpose(0, 1, 3, 2)  # -> [c, e, f, t]

    in_maps = [
        {"x": np.ascontiguousarray(xg[c]), "w": w_pack} for c in range(N_CORES)
    ]
    res = run_bass_kernel_spmd(nc, in_maps, list(range(N_CORES)), trace=trace)

    out = np.empty((B, IN_STRIDE), dtype=np.float32)
    for c in range(N_CORES):
        yc = res.results[c]["y"]  # [E, 512, cap]
        for e in range(E):
            idx = chunks[c][e]
            if len(idx):
                out[idx] = yc[e, :, : len(idx)].T
    return out, res


def kernel(tensor_in, tensor_w, tensor_w_id):
    out, _ = _run(tensor_in, tensor_w, tensor_w_id)
    return out
